# revision 23
# baseline (speedup 1.0000x reference)
"""Sparse-attention Bass kernel for 8 TRN2 NeuronCores.

Sharding: query-row parallel. Core c owns query rows [c*512, (c+1)*512) of
BOTH batch elements. The [n, n] mask is row-sharded (each 512-row slice read
once per core, reused across b and heads), K/V are computed redundantly per
core from the full batch (cheap vs. attention itself).

Layout trick: everything that needs a transpose (batch^T for the QKV matmuls,
mask^T for the attention layout) is pre-transposed and bf16-cast on the host
during sharding, so the device never transposes anything.

Per-core device pipeline, per (b, head-group g of 4 heads):
  S^T[j,i]   = K^T(g)[d,jblk]ᵀ-packed matmul vs Q^T(g) (4 heads concurrently
               via tile_position row bands, K=32 each)
  E^T        = exp(scale * S^T)           (one ACT instr per 2 heads, PSUM src)
  Em^T       = E^T * mask^T[jblk]         (DVE, bf16)
  U^T/rowsum = [V_h | 1]ᵀ @ Em_h^T        (PE, M=33, PSUM-accumulated over jblk)
  pre^T      = U^T * (1/rowsum)           (DVE recip + gpsimd bcast + DVE mul)
  out        = pre^T-packed matmul vs w_proj (K=dim, full-util, direct [i, c]
               layout -> contiguous DMA out)
"""

import numpy as np
from contextlib import ExitStack

import concourse.bass as bass
import concourse.tile as tile
from concourse import bacc, mybir
from concourse.bass_utils import run_bass_kernel_spmd

BF16 = mybir.dt.bfloat16
F32 = mybir.dt.float32
NPBF16 = mybir.dt.np(BF16)

B, N, DIM, H, D = 2, 4096, 512, 16, 32
NCORES = 8
NQ = N // NCORES            # query rows per core per batch elem (512)
G = 4                       # head groups (4 heads each)
HG = H // G                 # heads per group (4)
JB = N // 128               # key blocks (32)
SCALE = float(D) ** -0.5

# fast-exp (Schraudolph in bf16 bits): bits = round(A_EXP*y + B16) as uint16,
# saturating at 0 for masked (mb = -40000) entries. W_q is pre-scaled by
# A_EXP*SCALE so PSUM holds A_EXP*y directly; the ACT path compensates with
# scale=1/A_EXP and bias=LN_CORR (the fast-exp path's mean ratio 1.0407, so
# both paths carry the same constant factor, which cancels in softmax).
A_EXP = float(2 ** 7) / float(np.log(2.0))
B16 = 16256.0
MB_NEG = -40000.0
LN_CORR = 0.03988
# per-iteration (tile-pair) path routing: B = DVE fast-exp, G = ACT exp +
# gpsimd mask-mul, D = ACT exp + DVE mask-mul
PAIRS = ['BD', 'GD', 'BD', 'GD', 'BD']

_CACHE = {}


def build_nc():
    nc = bacc.Bacc("TRN2", target_bir_lowering=False, debug=False)

    batT = nc.declare_dram_parameter("batt", [B, DIM, N], BF16, isOutput=False)
    qrT = nc.declare_dram_parameter("qrt", [B, DIM, NQ], BF16, isOutput=False)
    wqkv = nc.declare_dram_parameter("wqkv", [DIM, 3 * DIM], BF16, isOutput=False)
    wproj = nc.declare_dram_parameter("wproj", [DIM, DIM], BF16, isOutput=False)
    maskT = nc.declare_dram_parameter("maskt", [N, NQ], mybir.dt.uint8, isOutput=False)
    mbT = nc.declare_dram_parameter("mbt", [N, NQ], BF16, isOutput=False)
    out = nc.declare_dram_parameter("out", [B, NQ, DIM], F32, isOutput=True)
    U16 = mybir.dt.uint16

    Exp = mybir.ActivationFunctionType.Exp

    with tile.TileContext(nc) as tc, ExitStack() as ctx:
        persist = ctx.enter_context(tc.tile_pool(name="persist", bufs=1))
        bpool = ctx.enter_context(tc.tile_pool(name="bpool", bufs=1))
        esbp = ctx.enter_context(tc.tile_pool(name="esbp", bufs=6))
        small = ctx.enter_context(tc.tile_pool(name="small", bufs=2))
        outp = ctx.enter_context(tc.tile_pool(name="outp", bufs=2))

        # ---- persistent loads -------------------------------------------
        wq_sb = []
        for k in range(4):
            t = persist.tile([128, 3 * DIM], BF16, tag=f"wqkv{k}")
            nc.sync.dma_start(out=t, in_=wqkv[k * 128:(k + 1) * 128, :])
            wq_sb.append(t)
        wp_sb = []
        for k in range(4):
            t = persist.tile([128, DIM], BF16, tag=f"wproj{k}")
            nc.sync.dma_start(out=t, in_=wproj[k * 128:(k + 1) * 128, :])
            wp_sb.append(t)
        mask_sb = []
        for jb in range(JB):
            t = persist.tile([128, NQ], mybir.dt.uint8, tag=f"mask{jb}")
            nc.sync.dma_start(out=t, in_=maskT[jb * 128:(jb + 1) * 128, :])
            mask_sb.append(t)
        mb_sb = []
        for jb in range(JB):
            t = persist.tile([128, NQ], BF16, tag=f"mb{jb}")
            nc.sync.dma_start(out=t, in_=mbT[jb * 128:(jb + 1) * 128, :])
            mb_sb.append(t)
        bias_sb = persist.tile([128, 1], F32, tag="biasc")
        nc.vector.memset(bias_sb, LN_CORR)

        for b in range(B):
            # ---- QKV phase ----------------------------------------------
            batT_sb = []
            for k in range(4):
                t = bpool.tile([128, N], BF16, tag=f"batT{k}")
                nc.sync.dma_start(out=t, in_=batT[b, k * 128:(k + 1) * 128, :])
                batT_sb.append(t)
            qrT_sb = []
            for k in range(4):
                t = bpool.tile([128, NQ], BF16, tag=f"qrT{k}")
                nc.sync.dma_start(out=t, in_=qrT[b, k * 128:(k + 1) * 128, :])
                qrT_sb.append(t)

            def make_kt_chunk(pool, t, g, jc):
                ps = pool.tile([128, 512], F32, tag="xps")
                for k in range(4):
                    nc.tensor.matmul(
                        ps,
                        wq_sb[k][:, DIM + 128 * g: DIM + 128 * g + 128],
                        batT_sb[k][:, jc * 512:(jc + 1) * 512],
                        start=(k == 0), stop=(k == 3),
                    )
                nc.vector.tensor_copy(t[:, jc * 512:(jc + 1) * 512], ps)

            def make_qt(pool, t, g):
                ps = pool.tile([128, 512], F32, tag="xps")
                for k in range(4):
                    nc.tensor.matmul(
                        ps,
                        wq_sb[k][:, 128 * g: 128 * g + 128],
                        qrT_sb[k],
                        start=(k == 0), stop=(k == 3),
                    )
                nc.vector.tensor_copy(t, ps)

            def make_v(pool, t, nb):
                ps = pool.tile([128, 512], F32, tag="xps")
                for k in range(4):
                    nc.tensor.matmul(
                        ps,
                        batT_sb[k][:, nb * 128:(nb + 1) * 128],
                        wq_sb[k][:, 2 * DIM: 3 * DIM],
                        start=(k == 0), stop=(k == 3),
                    )
                dst = bass.AP(
                    tensor=t.tensor, offset=t.offset,
                    ap=[t.ap[0], [33, H], [1, D]],
                )
                nc.scalar.copy(dst, ps)
                ones = bass.AP(
                    tensor=t.tensor, offset=t.offset + D,
                    ap=[t.ap[0], [33, H]],
                )
                nc.vector.memset(ones, 1.0)

            kt_sb = [bpool.tile([128, N], BF16, tag=f"kt{g}", name=f"kt{g}")
                     for g in range(G)]
            qt_sb = [bpool.tile([128, NQ], BF16, tag=f"qt{g}", name=f"qt{g}")
                     for g in range(G)]
            v_sb = [bpool.tile([128, H * (D + 1)], BF16, tag=f"v{nb}", name=f"v{nb}")
                    for nb in range(JB)]
            with tc.tile_pool(name=f"mm{b}", bufs=2, space="PSUM") as mm_ps:
                for jc in range(N // 512):
                    make_kt_chunk(mm_ps, kt_sb[0], 0, jc)
                make_qt(mm_ps, qt_sb[0], 0)
                make_v(mm_ps, v_sb[0], 0)

            # ---- attention ----------------------------------------------
            with (tc.tile_pool(name=f"st{b}", bufs=2, space="PSUM") as st_ps,
                  tc.tile_pool(name=f"avp{b}", bufs=1, space="PSUM") as av_ps,
                  tc.tile_pool(name=f"xtr{b}", bufs=2, space="PSUM") as xtr_ps):
                pre_sb = []
                for g in range(G):
                    av = av_ps.tile([128, 1024], F32, tag="av")
                    for jb in range(JB):
                        st = st_ps.tile([128, 1024], F32, tag="st")
                        st2 = st_ps.tile([128, 1024], F32, tag="st")
                        for r in range(HG):
                            dst = (st if r < 2 else st2)[:, (r % 2) * 512:(r % 2) * 512 + 512]
                            nc.tensor.matmul(
                                dst,
                                kt_sb[g][32 * r:32 * r + 32, jb * 128:(jb + 1) * 128],
                                qt_sb[g][32 * r:32 * r + 32, :],
                                start=True, stop=True,
                                tile_position=(32 * r, 0),
                            )
                        # deferred QKV work, emitted after the score quad so
                        # the 4 banded matmuls issue back-to-back
                        if g == 0:
                            if jb < 16:
                                make_kt_chunk(xtr_ps, kt_sb[1 + jb // 8], 1 + jb // 8, jb % 8)
                            elif jb == 16:
                                make_qt(xtr_ps, qt_sb[1], 1)
                            if jb < JB - 1:
                                make_v(xtr_ps, v_sb[jb + 1], jb + 1)
                        elif g == 1:
                            if jb < 8:
                                make_kt_chunk(xtr_ps, kt_sb[3], 3, jb)
                            elif jb == 8:
                                make_qt(xtr_ps, qt_sb[2], 2)
                            elif jb == 9:
                                make_qt(xtr_ps, qt_sb[3], 3)
                        e = esbp.tile([128, 1024], BF16, tag="e")
                        e2 = esbp.tile([128, 1024], BF16, tag="e")
                        mrep = bass.AP(
                            tensor=mask_sb[jb].tensor, offset=mask_sb[jb].offset,
                            ap=[mask_sb[jb].ap[0], [0, 2], [1, 512]],
                        )
                        mbrep = bass.AP(
                            tensor=mb_sb[jb].tensor, offset=mb_sb[jb].offset,
                            ap=[mb_sb[jb].ap[0], [0, 2], [1, 512]],
                        )
                        # pair pattern: each iteration drains one tile on DVE
                        # and one on ACT; gpsimd muls never pair with B (so a
                        # single engine is never the serial gate for a pair)
                        pair = PAIRS[((b * G + g) * JB + jb) % len(PAIRS)]
                        for half, (ee, ss) in enumerate(((e, st), (e2, st2))):
                            path = pair[half]
                            if path == 'B':
                                # fast-exp: bits = u16(A*y + mb), masked -> 0
                                nc.vector.tensor_tensor(
                                    ee.bitcast(U16), ss, mbrep,
                                    mybir.AluOpType.add)
                            else:
                                nc.scalar.activation(
                                    ee, ss, Exp, scale=1.0 / A_EXP, bias=bias_sb)
                                if path == 'G':
                                    nc.gpsimd.tensor_tensor(
                                        ee, ee, mrep, mybir.AluOpType.mult)
                                else:
                                    nc.vector.tensor_mul(ee, ee, mrep)
                        for r in range(HG):
                            esl = (e if r < 2 else e2)[:, (r % 2) * 512:(r % 2) * 512 + 512]
                            h = g * HG + r
                            nc.tensor.matmul(
                                av[64 * (r % 2):64 * (r % 2) + 33,
                                   (r // 2) * 512:(r // 2) * 512 + 512],
                                v_sb[jb][:, 33 * h: 33 * h + 33],
                                esl,
                                start=(jb == 0), stop=(jb == JB - 1),
                                tile_position=(0, 64 * (r % 2)),
                            )
                    # normalize -> pre^T [128 (4h x 32d), NQ] bf16
                    # gather the 4 rowsum rows (psum partitions {32,96} x 2 free
                    # halves) into one [2,1024] tile, one reciprocal, then
                    # partition-broadcast each row via SBUF->SBUF DMA
                    # reuse the first NQ columns of kt (dead after group g's
                    # scores) as the pre^T staging buffer
                    pre = kt_sb[g]
                    for r in range(HG):
                        pb = 64 * (r % 2)
                        fo = (r // 2) * 512
                        # stage rowsum at partition base 0 (PSUM->SB cross-base
                        # copy is legal; approx_fast needs a base-0 source)
                        rsr = small.tile([1, NQ], F32, tag="rsr")
                        nc.vector.tensor_copy(rsr, av[pb + 32: pb + 33, fo: fo + 512])
                        rcp = small.tile([1, NQ], F32, tag="rcp")
                        nc.vector.reciprocal_approx_fast(rcp, rsr)
                        rcpb = small.tile([32, NQ], F32, tag="rcpb")
                        nc.gpsimd.partition_broadcast(rcpb, rcp[0:1, :], channels=32)
                        nc.vector.tensor_mul(
                            pre[32 * r: 32 * r + 32, 0:NQ],
                            av[pb: pb + 32, fo: fo + 512],
                            rcpb,
                        )
                    pre_sb.append(pre)

            # ---- output projection --------------------------------------
            with tc.tile_pool(name=f"pj{b}", bufs=2, space="PSUM") as mm_ps:
                for ib in range(NQ // 128):
                    ps = mm_ps.tile([128, DIM], F32)
                    for g in range(G):
                        nc.tensor.matmul(
                            ps,
                            pre_sb[g][:, ib * 128:(ib + 1) * 128],
                            wp_sb[g],
                            start=(g == 0), stop=(g == 3),
                        )
                    o = outp.tile([128, DIM], F32, tag="o")
                    nc.vector.tensor_copy(o, ps)
                    nc.sync.dma_start(out=out[b, ib * 128:(ib + 1) * 128, :], in_=o)

    nc.compile()
    return nc


def _prep_inputs(batch, w_qkv, w_proj, custom_mask):
    batch = np.asarray(batch, np.float32)
    w_scaled = np.array(np.asarray(w_qkv, np.float32))
    w_scaled[:, :DIM] *= A_EXP * SCALE      # fold exp input scaling into W_q
    wqkv_bf = w_scaled.astype(NPBF16)
    wproj_bf = np.asarray(w_proj, np.float32).astype(NPBF16)
    batT = np.ascontiguousarray(batch.transpose(0, 2, 1)).astype(NPBF16)
    m = np.asarray(custom_mask, np.float32)[0, 0]  # [N, N] 0/1
    in_maps = []
    for c in range(NCORES):
        rows = slice(c * NQ, (c + 1) * NQ)
        qrT = np.ascontiguousarray(batch[:, rows, :].transpose(0, 2, 1)).astype(NPBF16)
        mT = np.ascontiguousarray(m[rows, :].T)
        mbT = np.where(mT > 0, B16, MB_NEG).astype(NPBF16)
        in_maps.append({
            "batt": batT, "qrt": qrT, "wqkv": wqkv_bf,
            "wproj": wproj_bf, "maskt": mT.astype(np.uint8), "mbt": mbT,
        })
    return in_maps


def _run(in_maps, trace=False, **kw):
    if "nc" not in _CACHE:
        _CACHE["nc"] = build_nc()
    return run_bass_kernel_spmd(
        _CACHE["nc"], in_maps, core_ids=list(range(NCORES)), trace=trace, **kw
    )


def kernel(batch, w_qkv, w_proj, custom_mask):
    in_maps = _prep_inputs(batch, w_qkv, w_proj, custom_mask)
    res = _run(in_maps)
    full = np.empty((B, N, DIM), np.float32)
    for c in range(NCORES):
        full[:, c * NQ:(c + 1) * NQ, :] = res.results[c]["out"]
    return full



# revision 29
# speedup vs baseline: 1.0875x; 1.0875x over previous
"""Sparse-attention Bass kernel for 8 TRN2 NeuronCores.

Sharding: query-row parallel. Core c owns query rows [c*512, (c+1)*512) of
BOTH batch elements. The [n, n] mask is row-sharded (each 512-row slice read
once per core, reused across b and heads), K/V are computed redundantly per
core from the full batch (cheap vs. attention itself).

Layout trick: everything that needs a transpose (batch^T for the QKV matmuls,
mask^T for the attention layout) is pre-transposed and bf16-cast on the host
during sharding, so the device never transposes anything.

Per-core device pipeline, per (b, head-group g of 4 heads):
  S^T[j,i]   = K^T(g)[d,jblk]ᵀ-packed matmul vs Q^T(g) (4 heads concurrently
               via tile_position row bands, K=32 each)
  E^T        = exp(scale * S^T)           (one ACT instr per 2 heads, PSUM src)
  Em^T       = E^T * mask^T[jblk]         (DVE, bf16)
  U^T/rowsum = [V_h | 1]ᵀ @ Em_h^T        (PE, M=33, PSUM-accumulated over jblk)
  pre^T      = U^T * (1/rowsum)           (DVE recip + gpsimd bcast + DVE mul)
  out        = pre^T-packed matmul vs w_proj (K=dim, full-util, direct [i, c]
               layout -> contiguous DMA out)
"""

import numpy as np
from contextlib import ExitStack

import concourse.bass as bass
import concourse.tile as tile
from concourse import bacc, mybir
from concourse.bass_utils import run_bass_kernel_spmd

BF16 = mybir.dt.bfloat16
F32 = mybir.dt.float32
NPBF16 = mybir.dt.np(BF16)

B, N, DIM, H, D = 2, 4096, 512, 16, 32
NCORES = 8
NQ = N // NCORES            # query rows per core per batch elem (512)
G = 4                       # head groups (4 heads each)
HG = H // G                 # heads per group (4)
JB = N // 128               # key blocks (32)
SCALE = float(D) ** -0.5

# fast-exp (Schraudolph in bf16 bits): bits = round(A_EXP*y + B16) as uint16,
# saturating at 0 for masked (mb = -40000) entries. W_q is pre-scaled by
# A_EXP*SCALE so PSUM holds A_EXP*y directly; the ACT path compensates with
# scale=1/A_EXP and bias=LN_CORR (the fast-exp path's mean ratio 1.0407, so
# both paths carry the same constant factor, which cancels in softmax).
A_EXP = float(2 ** 7) / float(np.log(2.0))
B16 = 16256.0
MB_NEG = -40000.0
LN_CORR = 0.03988
# per-head-tile path routing: B = DVE fast-exp, G = ACT exp + gpsimd
# mask-mul, D = ACT exp + DVE mask-mul (STT indicator trick on mb)
QUADS = ['BDBD', 'BDGD']

_CACHE = {}


def build_nc():
    nc = bacc.Bacc("TRN2", target_bir_lowering=False, debug=False)

    batT = nc.declare_dram_parameter("batt", [B, DIM, N], BF16, isOutput=False)
    qrT = nc.declare_dram_parameter("qrt", [B, DIM, NQ], BF16, isOutput=False)
    wqkv = nc.declare_dram_parameter("wqkv", [DIM, 3 * DIM], BF16, isOutput=False)
    wproj = nc.declare_dram_parameter("wproj", [DIM, DIM], BF16, isOutput=False)
    maskT = nc.declare_dram_parameter("maskt", [N, NQ], mybir.dt.uint8, isOutput=False)
    mbT = nc.declare_dram_parameter("mbt", [N, NQ], BF16, isOutput=False)
    out = nc.declare_dram_parameter("out", [B, NQ, DIM], F32, isOutput=True)
    U16 = mybir.dt.uint16

    Exp = mybir.ActivationFunctionType.Exp

    with tile.TileContext(nc) as tc, ExitStack() as ctx:
        persist = ctx.enter_context(tc.tile_pool(name="persist", bufs=1))
        bpool = ctx.enter_context(tc.tile_pool(name="bpool", bufs=1))
        esbp = ctx.enter_context(tc.tile_pool(name="esbp", bufs=6))
        small = ctx.enter_context(tc.tile_pool(name="small", bufs=2))
        outp = ctx.enter_context(tc.tile_pool(name="outp", bufs=2))

        # ---- persistent loads -------------------------------------------
        wq_sb = []
        for k in range(4):
            t = persist.tile([128, 3 * DIM], BF16, tag=f"wqkv{k}")
            nc.sync.dma_start(out=t, in_=wqkv[k * 128:(k + 1) * 128, :])
            wq_sb.append(t)
        wp_sb = []
        for k in range(4):
            t = persist.tile([128, DIM], BF16, tag=f"wproj{k}")
            nc.sync.dma_start(out=t, in_=wproj[k * 128:(k + 1) * 128, :])
            wp_sb.append(t)
        mask_sb = []
        for jb in range(JB):
            t = persist.tile([128, NQ], mybir.dt.uint8, tag=f"mask{jb}")
            nc.sync.dma_start(out=t, in_=maskT[jb * 128:(jb + 1) * 128, :])
            mask_sb.append(t)
        mb_sb = []
        for jb in range(JB):
            t = persist.tile([128, NQ], BF16, tag=f"mb{jb}")
            nc.sync.dma_start(out=t, in_=mbT[jb * 128:(jb + 1) * 128, :])
            mb_sb.append(t)
        bias_sb = persist.tile([128, 1], F32, tag="biasc")
        nc.vector.memset(bias_sb, LN_CORR)

        for b in range(B):
            # ---- QKV phase ----------------------------------------------
            batT_sb = []
            for k in range(4):
                t = bpool.tile([128, N], BF16, tag=f"batT{k}")
                nc.sync.dma_start(out=t, in_=batT[b, k * 128:(k + 1) * 128, :])
                batT_sb.append(t)
            qrT_sb = []
            for k in range(4):
                t = bpool.tile([128, NQ], BF16, tag=f"qrT{k}")
                nc.sync.dma_start(out=t, in_=qrT[b, k * 128:(k + 1) * 128, :])
                qrT_sb.append(t)

            def make_kt_chunk(pool, t, g, jc, tag="xps"):
                ps = pool.tile([128, 512], F32, tag=tag)
                for k in range(4):
                    nc.tensor.matmul(
                        ps,
                        wq_sb[k][:, DIM + 128 * g: DIM + 128 * g + 128],
                        batT_sb[k][:, jc * 512:(jc + 1) * 512],
                        start=(k == 0), stop=(k == 3),
                    )
                nc.vector.tensor_copy(t[:, jc * 512:(jc + 1) * 512], ps)

            def make_qt(pool, t, g, tag="xps"):
                ps = pool.tile([128, 512], F32, tag=tag)
                for k in range(4):
                    nc.tensor.matmul(
                        ps,
                        wq_sb[k][:, 128 * g: 128 * g + 128],
                        qrT_sb[k],
                        start=(k == 0), stop=(k == 3),
                    )
                nc.vector.tensor_copy(t, ps)

            def make_v(pool, t, nb, tag="xps"):
                ps = pool.tile([128, 512], F32, tag=tag)
                for k in range(4):
                    nc.tensor.matmul(
                        ps,
                        batT_sb[k][:, nb * 128:(nb + 1) * 128],
                        wq_sb[k][:, 2 * DIM: 3 * DIM],
                        start=(k == 0), stop=(k == 3),
                    )
                dst = bass.AP(
                    tensor=t.tensor, offset=t.offset,
                    ap=[t.ap[0], [33, H], [1, D]],
                )
                nc.scalar.copy(dst, ps)
                ones = bass.AP(
                    tensor=t.tensor, offset=t.offset + D,
                    ap=[t.ap[0], [33, H]],
                )
                nc.vector.memset(ones, 1.0)

            kt_sb = [bpool.tile([128, N], BF16, tag=f"kt{g}", name=f"kt{g}")
                     for g in range(G)]
            qt_sb = [bpool.tile([128, NQ], BF16, tag=f"qt{g}", name=f"qt{g}")
                     for g in range(G)]
            v_sb = [bpool.tile([128, H * (D + 1)], BF16, tag=f"v{nb}", name=f"v{nb}")
                    for nb in range(JB)]
            with tc.tile_pool(name=f"mm{b}", bufs=2, space="PSUM") as mm_ps:
                for jc in range(N // 512):
                    make_kt_chunk(mm_ps, kt_sb[0], 0, jc)
                make_qt(mm_ps, qt_sb[0], 0)
                make_v(mm_ps, v_sb[0], 0)

            # ---- attention ----------------------------------------------
            with (tc.tile_pool(name=f"st{b}", bufs=6, space="PSUM") as st_ps,
                  tc.tile_pool(name=f"avp{b}", bufs=1, space="PSUM") as av_ps):
                xtr_ps = st_ps  # deferred-make chunks share the st rotation
                pre_sb = []
                for g in range(G):
                    av = av_ps.tile([128, 1024], F32, tag="av")
                    for jb in range(JB):
                        sts = [st_ps.tile([128, 512], F32, tag="st",
                                          name=f"st{r}")
                               for r in range(HG)]
                        for r in range(HG):
                            nc.tensor.matmul(
                                sts[r],
                                kt_sb[g][32 * r:32 * r + 32, jb * 128:(jb + 1) * 128],
                                qt_sb[g][32 * r:32 * r + 32, :],
                                start=True, stop=True,
                                tile_position=(32 * r, 0),
                            )
                        # deferred QKV work, emitted after the score quad so
                        # the 4 banded matmuls issue back-to-back
                        if g == 0:
                            if jb < 16:
                                make_kt_chunk(xtr_ps, kt_sb[1 + jb // 8], 1 + jb // 8, jb % 8, tag="st")
                            elif jb == 16:
                                make_qt(xtr_ps, qt_sb[1], 1, tag="st")
                            if jb < JB - 1:
                                make_v(xtr_ps, v_sb[jb + 1], jb + 1, tag="st")
                        elif g == 1:
                            if jb < 8:
                                make_kt_chunk(xtr_ps, kt_sb[3], 3, jb, tag="st")
                            elif jb == 8:
                                make_qt(xtr_ps, qt_sb[2], 2, tag="st")
                            elif jb == 9:
                                make_qt(xtr_ps, qt_sb[3], 3, tag="st")
                        es = [esbp.tile([128, 512], BF16, tag="e",
                                        name=f"e{r}")
                              for r in range(HG)]
                        # per-head drains spread across DVE/ACT/gpsimd; B
                        # (fast-exp) and D both read only the mb tensor
                        quad = QUADS[((b * G + g) * JB + jb) % len(QUADS)]
                        for r in range(HG):
                            ee, ss, path = es[r], sts[r], quad[r]
                            if path == 'B':
                                # fast-exp: bits = u16(A*y + mb), masked -> 0
                                nc.vector.tensor_tensor(
                                    ee.bitcast(U16), ss, mb_sb[jb],
                                    mybir.AluOpType.add)
                            else:
                                nc.scalar.activation(
                                    ee, ss, Exp, scale=1.0 / A_EXP, bias=bias_sb)
                                if path == 'G':
                                    nc.gpsimd.tensor_tensor(
                                        ee, ee, mask_sb[jb], mybir.AluOpType.mult)
                                else:
                                    # e *= (mb > 0), one fused DVE op
                                    nc.vector.scalar_tensor_tensor(
                                        ee, mb_sb[jb], 0.0, ee,
                                        mybir.AluOpType.is_gt,
                                        mybir.AluOpType.mult)
                        for r in range(HG):
                            h = g * HG + r
                            nc.tensor.matmul(
                                av[64 * (r % 2):64 * (r % 2) + 33,
                                   (r // 2) * 512:(r // 2) * 512 + 512],
                                v_sb[jb][:, 33 * h: 33 * h + 33],
                                es[r],
                                start=(jb == 0), stop=(jb == JB - 1),
                                tile_position=(0, 64 * (r % 2)),
                            )
                    # normalize -> pre^T [128 (4h x 32d), NQ] bf16
                    # gather the 4 rowsum rows (psum partitions {32,96} x 2 free
                    # halves) into one [2,1024] tile, one reciprocal, then
                    # partition-broadcast each row via SBUF->SBUF DMA
                    # reuse the first NQ columns of kt (dead after group g's
                    # scores) as the pre^T staging buffer
                    pre = kt_sb[g]
                    for r in range(HG):
                        pb = 64 * (r % 2)
                        fo = (r // 2) * 512
                        # stage rowsum at partition base 0 (PSUM->SB cross-base
                        # copy is legal; approx_fast needs a base-0 source)
                        rsr = small.tile([1, NQ], F32, tag="rsr")
                        nc.vector.tensor_copy(rsr, av[pb + 32: pb + 33, fo: fo + 512])
                        rcp = small.tile([1, NQ], F32, tag="rcp")
                        nc.vector.reciprocal_approx_fast(rcp, rsr)
                        rcpb = small.tile([32, NQ], F32, tag="rcpb")
                        nc.gpsimd.partition_broadcast(rcpb, rcp[0:1, :], channels=32)
                        nc.vector.tensor_mul(
                            pre[32 * r: 32 * r + 32, 0:NQ],
                            av[pb: pb + 32, fo: fo + 512],
                            rcpb,
                        )
                    pre_sb.append(pre)

            # ---- output projection --------------------------------------
            with tc.tile_pool(name=f"pj{b}", bufs=2, space="PSUM") as mm_ps:
                for ib in range(NQ // 128):
                    ps = mm_ps.tile([128, DIM], F32)
                    for g in range(G):
                        nc.tensor.matmul(
                            ps,
                            pre_sb[g][:, ib * 128:(ib + 1) * 128],
                            wp_sb[g],
                            start=(g == 0), stop=(g == 3),
                        )
                    o = outp.tile([128, DIM], F32, tag="o")
                    nc.vector.tensor_copy(o, ps)
                    nc.sync.dma_start(out=out[b, ib * 128:(ib + 1) * 128, :], in_=o)

    nc.compile()
    return nc


def _prep_inputs(batch, w_qkv, w_proj, custom_mask):
    batch = np.asarray(batch, np.float32)
    w_scaled = np.array(np.asarray(w_qkv, np.float32))
    w_scaled[:, :DIM] *= A_EXP * SCALE      # fold exp input scaling into W_q
    wqkv_bf = w_scaled.astype(NPBF16)
    wproj_bf = np.asarray(w_proj, np.float32).astype(NPBF16)
    batT = np.ascontiguousarray(batch.transpose(0, 2, 1)).astype(NPBF16)
    m = np.asarray(custom_mask, np.float32)[0, 0]  # [N, N] 0/1
    in_maps = []
    for c in range(NCORES):
        rows = slice(c * NQ, (c + 1) * NQ)
        qrT = np.ascontiguousarray(batch[:, rows, :].transpose(0, 2, 1)).astype(NPBF16)
        mT = np.ascontiguousarray(m[rows, :].T)
        mbT = np.where(mT > 0, B16, MB_NEG).astype(NPBF16)
        in_maps.append({
            "batt": batT, "qrt": qrT, "wqkv": wqkv_bf,
            "wproj": wproj_bf, "maskt": mT.astype(np.uint8), "mbt": mbT,
        })
    return in_maps


def _run(in_maps, trace=False, **kw):
    if "nc" not in _CACHE:
        _CACHE["nc"] = build_nc()
    return run_bass_kernel_spmd(
        _CACHE["nc"], in_maps, core_ids=list(range(NCORES)), trace=trace, **kw
    )


def kernel(batch, w_qkv, w_proj, custom_mask):
    in_maps = _prep_inputs(batch, w_qkv, w_proj, custom_mask)
    res = _run(in_maps)
    full = np.empty((B, N, DIM), np.float32)
    for c in range(NCORES):
        full[:, c * NQ:(c + 1) * NQ, :] = res.results[c]["out"]
    return full

